# revision 1
# baseline (speedup 1.0000x reference)
"""Trainium2 Bass kernel for a 4-layer LIF spiking net (BPSpikingNet).

Reference semantics (per timestep t, per layer l):
    i = h @ W_l.T + b_l
    v = v - v/tau + i          (tau=2  ->  v = 0.5*v + i)
    s = (v >= 1.0)
    v = (1-s) * v              (hard reset to 0)
    h = s
Output = layer-4 spike train, shape [T=32, B=128, 1000], fp32.

Strategy:
  * Data-parallel over batch: B=128 -> 16 samples per core across 8 cores.
  * Layer-by-layer: layer l's matmul input (spikes of l-1) is fully known
    once l-1's recurrence is done, so each layer is ONE dense GEMM over all
    T*Bs = 512 (t,b) columns (neuron-major / weight-stationary, N=512 moving),
    followed by a 32-step elementwise LIF recurrence on [128, O*16] tiles.
  * bf16 matmuls (spikes are exact in bf16; weight rounding is far below the
    spiking threshold margin), fp32 PSUM accumulate, fp32 recurrence.
"""

import numpy as np
import ml_dtypes

T = 32
B = 128
NCORES = 8
BS = B // NCORES          # 16 samples per core
COLS = T * BS             # 512 (t,b) columns per core
NIN = 2048
KT = NIN // 128           # 16 k-tiles (all layers have 2048 inputs)
O_LIST = [16, 16, 16, 8]  # output 128-tiles per layer (layer 4 padded 1000->1024)
BOFF = [0, 16, 32, 48]    # bias column offset per layer
NB = sum(O_LIST)          # 56 bias columns

_CACHE = {}

TRACE = False             # set True (from test.py) to capture an NTFF profile
LAST_RESULTS = None       # BassKernelResults of the most recent run


def _build_nc():
    import concourse.mybir as mybir
    import concourse.tile as tile
    from concourse import bacc

    dt = mybir.dt
    alu = mybir.AluOpType

    nc = bacc.Bacc("TRN2", target_bir_lowering=False, debug=False,
                   num_devices=NCORES)

    x_d = nc.dram_tensor("x", [128, KT, COLS], dt.bfloat16, kind="ExternalInput")
    w_d = [
        nc.dram_tensor(f"w{li}", [O_LIST[li], 128, KT, 128], dt.bfloat16,
                       kind="ExternalInput")
        for li in range(4)
    ]
    b_d = nc.dram_tensor("bias", [128, NB], dt.float32, kind="ExternalInput")
    out_d = nc.dram_tensor("out", [128, O_LIST[3], COLS], dt.float32,
                           kind="ExternalOutput")

    with tile.TileContext(nc) as tc:
        with (
            tc.tile_pool(name="xp", bufs=1) as xp,
            tc.tile_pool(name="sp", bufs=1) as sp,
            tc.tile_pool(name="ip", bufs=1) as ip,
            tc.tile_pool(name="wp", bufs=4) as wp,
            tc.tile_pool(name="vp", bufs=1) as vp,
            tc.tile_pool(name="bp", bufs=1) as bp,
            tc.tile_pool(name="ps", bufs=4, space="PSUM") as ps,
        ):
            xt = xp.tile([128, KT, COLS], dt.bfloat16)
            nc.sync.dma_start(xt[:], x_d.ap())
            bt = bp.tile([128, NB], dt.float32)
            nc.sync.dma_start(bt[:], b_d.ap())

            rhs = xt
            st = None
            for li in range(4):
                O = O_LIST[li]
                # current buffer [128, t, o, b]; one shared slot reused across layers
                it = ip.tile([128, T, O, BS], dt.float32, tag="it")
                for o in range(O):
                    wt = wp.tile([128, KT, 128], dt.bfloat16, tag="wt")
                    nc.sync.dma_start(wt[:], w_d[li].ap()[o])
                    acc = ps.tile([128, COLS], dt.float32, tag="acc")
                    for k in range(KT):
                        nc.tensor.matmul(acc[:], wt[:, k, :], rhs[:, k, :],
                                         start=(k == 0), stop=(k == KT - 1))
                    # PSUM -> SBUF eviction with bias add, scattered to t-major
                    nc.vector.tensor_scalar(
                        it[:, :, o, :],
                        acc.rearrange("p (t b) -> p t b", t=T),
                        bt[:, BOFF[li] + o:BOFF[li] + o + 1],
                        None,
                        alu.add,
                    )
                # LIF recurrence over T steps on [128, O, BS]
                va = vp.tile([128, O, BS], dt.float32, tag=f"va{li}")
                vb = vp.tile([128, O, BS], dt.float32, tag=f"vb{li}")
                nc.vector.memset(vb[:], 0.0)
                sdt = dt.bfloat16 if li < 3 else dt.float32
                st = sp.tile([128, O, COLS], sdt, tag=f"s{li}")
                for t in range(T):
                    # charge: va = 0.5*vb + i_t
                    nc.vector.scalar_tensor_tensor(
                        va[:], vb[:], 0.5, it[:, t], alu.mult, alu.add)
                    # spike: s_t = (va >= 1)  (gpsimd, off the DVE critical path)
                    nc.gpsimd.tensor_scalar(
                        st[:, :, t * BS:(t + 1) * BS], va[:], 1.0, None, alu.is_ge)
                    # reset: vb = (va < 1) * va
                    nc.vector.scalar_tensor_tensor(
                        vb[:], va[:], 1.0, va[:], alu.is_lt, alu.mult)
                rhs = st
            nc.sync.dma_start(out_d.ap(), st[:])

    nc.compile()
    return nc


def _get_nc():
    if "nc" not in _CACHE:
        _CACHE["nc"] = _build_nc()
    return _CACHE["nc"]


def _host_inputs(x_tbf, Ws, bs):
    """Shared (weight/bias) arrays + per-core x shards, pre-laid-out."""
    bf16 = ml_dtypes.bfloat16
    w_arrs = []
    b_cols = []
    for li in range(4):
        W = np.asarray(Ws[li], np.float32)
        b = np.asarray(bs[li], np.float32)
        O = O_LIST[li]
        if W.shape[0] < O * 128:           # pad layer 4: 1000 -> 1024
            pad = O * 128 - W.shape[0]
            W = np.concatenate([W, np.zeros((pad, NIN), np.float32)], 0)
            b = np.concatenate([b, np.zeros(pad, np.float32)])
        # warr[o, ki, k, mo] = W[o*128+mo, k*128+ki]
        w_arrs.append(np.ascontiguousarray(
            W.reshape(O, 128, KT, 128).transpose(0, 3, 2, 1)).astype(bf16))
        b_cols.append(b.reshape(O, 128))
    b_all = np.ascontiguousarray(np.concatenate(b_cols, 0).T).astype(np.float32)

    x = np.asarray(x_tbf, np.float32)
    x_shards = []
    for c in range(NCORES):
        xc = x[:, c * BS:(c + 1) * BS, :]                    # [T, BS, NIN]
        xc = xc.transpose(2, 0, 1).reshape(NIN, COLS)        # [n, t*BS+b]
        xc = xc.reshape(KT, 128, COLS).transpose(1, 0, 2)    # [p, k, cols]
        x_shards.append(np.ascontiguousarray(xc).astype(bf16))
    return w_arrs, b_all, x_shards


def kernel(x_tbf, W1, b1, W2, b2, W3, b3, W4, b4):
    global LAST_RESULTS
    from concourse.bass_utils import run_bass_kernel_spmd

    nc = _get_nc()
    w_arrs, b_all, x_shards = _host_inputs(
        x_tbf, [W1, W2, W3, W4], [b1, b2, b3, b4])

    in_maps = []
    for c in range(NCORES):
        m = {"x": x_shards[c], "bias": b_all}
        for li in range(4):
            m[f"w{li}"] = w_arrs[li]
        in_maps.append(m)

    res = run_bass_kernel_spmd(nc, in_maps, core_ids=list(range(NCORES)),
                               trace=TRACE)
    LAST_RESULTS = res

    out = np.empty((T, B, 1000), np.float32)
    for c in range(NCORES):
        oc = np.asarray(res.results[c]["out"], np.float32)   # [128, 8, COLS]
        oc = oc.transpose(1, 0, 2).reshape(O_LIST[3] * 128, T, BS)
        out[:, c * BS:(c + 1) * BS, :] = oc[:1000].transpose(1, 2, 0)
    return out


# revision 2
# speedup vs baseline: 2.1878x; 2.1878x over previous
"""Trainium2 Bass kernel for a 4-layer LIF spiking net (BPSpikingNet).

Reference semantics (per timestep t, per layer l):
    i = h @ W_l.T + b_l
    v = v - v/tau + i          (tau=2  ->  v = 0.5*v + i)
    s = (v >= 1.0)
    v = (1-s) * v              (hard reset to 0)
    h = s
Output = layer-4 spike train, shape [T=32, B=128, 1000], fp32.

Strategy:
  * Data-parallel over batch: B=128 -> 16 samples per core across 8 cores.
  * Layer-by-layer: layer l's matmul input (spikes of l-1) is fully known
    once l-1's recurrence is done, so each layer is ONE dense GEMM over all
    T*Bs = 512 (t,b) columns (neuron-major / weight-stationary, N=512 moving),
    followed by a 32-step elementwise LIF recurrence on [128, O*16] tiles.
  * bf16 matmuls (spikes are exact in bf16; weight rounding is far below the
    spiking threshold margin), fp32 PSUM accumulate, fp32 recurrence.
  * Recurrence: charge writes the charged potential in-place into the current
    buffer iT[:, t] (2 DVE ops per step on the serial chain), and spikes for
    ALL timesteps are extracted afterwards with a single big is_ge op.
"""

import numpy as np
import ml_dtypes

T = 32
B = 128
NCORES = 8
BS = B // NCORES          # 16 samples per core
COLS = T * BS             # 512 (t,b) columns per core
NIN = 2048
KT = NIN // 128           # 16 k-tiles (all layers have 2048 inputs)
O_LIST = [16, 16, 16, 8]  # output 128-tiles per layer (layer 4 padded 1000->1024)
BOFF = [0, 16, 32, 48]    # bias column offset per layer
NB = sum(O_LIST)          # 56 bias columns

_CACHE = {}

TRACE = False             # set True (from test.py) to capture an NTFF profile
LAST_RESULTS = None       # BassKernelResults of the most recent run
EVICT_ENGINE = "scalar"   # "scalar" (ACT Identity+bias) or "vector" fallback


def _build_nc():
    import concourse.mybir as mybir
    import concourse.tile as tile
    from concourse import bacc

    dt = mybir.dt
    alu = mybir.AluOpType

    nc = bacc.Bacc("TRN2", target_bir_lowering=False, debug=False,
                   num_devices=NCORES)

    x_d = nc.dram_tensor("x", [128, KT, COLS], dt.bfloat16, kind="ExternalInput")
    w_d = [
        nc.dram_tensor(f"w{li}", [O_LIST[li], 128, KT, 128], dt.bfloat16,
                       kind="ExternalInput")
        for li in range(4)
    ]
    b_d = nc.dram_tensor("bias", [128, NB], dt.float32, kind="ExternalInput")
    out_d = nc.dram_tensor("out", [128, T, O_LIST[3], BS], dt.bfloat16,
                           kind="ExternalOutput")

    with tile.TileContext(nc) as tc:
        with (
            tc.tile_pool(name="xp", bufs=1) as xp,
            tc.tile_pool(name="sp", bufs=1) as sp,
            tc.tile_pool(name="ip", bufs=1) as ip,
            tc.tile_pool(name="wp", bufs=6) as wp,
            tc.tile_pool(name="vp", bufs=1) as vp,
            tc.tile_pool(name="bp", bufs=1) as bp,
            tc.tile_pool(name="ps", bufs=4, space="PSUM") as ps,
        ):
            xt = xp.tile([128, KT, COLS], dt.bfloat16)
            nc.sync.dma_start(xt[:], x_d.ap())
            bt = bp.tile([128, NB], dt.float32)
            nc.sync.dma_start(bt[:], b_d.ap())

            st = None
            for li in range(4):
                O = O_LIST[li]
                # current/charged-potential buffer [128, t, o, b]; one shared
                # slot reused across layers (dead once spikes are extracted)
                it = ip.tile([128, T, O, BS], dt.float32, tag="it")
                for o in range(O):
                    wt = wp.tile([128, KT, 128], dt.bfloat16, tag="wt")
                    nc.sync.dma_start(wt[:], w_d[li].ap()[o])
                    acc = ps.tile([128, COLS], dt.float32, tag="acc")
                    for k in range(KT):
                        if li == 0:
                            rhs = xt[:, k, :]                  # contiguous
                        else:
                            rhs = st[:, :, k, :]               # t-major strided
                        nc.tensor.matmul(acc[:], wt[:, k, :], rhs,
                                         start=(k == 0), stop=(k == KT - 1))
                    # PSUM -> SBUF eviction with bias add, scattered to t-major
                    bias_ap = bt[:, BOFF[li] + o:BOFF[li] + o + 1]
                    src = acc.rearrange("p (t b) -> p t b", t=T)
                    if EVICT_ENGINE == "scalar":
                        nc.scalar.activation(
                            it[:, :, o, :], src,
                            mybir.ActivationFunctionType.Identity,
                            bias=bias_ap, scale=1.0)
                    else:
                        nc.vector.tensor_scalar(
                            it[:, :, o, :], src, bias_ap, None, alu.add)
                # LIF recurrence: charge in place (iT[:,t] becomes the charged
                # potential v(t)); only the reset state vb carries between steps
                vb = vp.tile([128, O, BS], dt.float32, tag=f"vb{li}")
                nc.vector.memset(vb[:], 0.0)
                for t in range(T):
                    # charge: iT[:,t] = 0.5*vb + iT[:,t]
                    nc.vector.scalar_tensor_tensor(
                        it[:, t], vb[:], 0.5, it[:, t], alu.mult, alu.add)
                    # reset: vb = (iT[:,t] < 1) * iT[:,t]
                    nc.vector.scalar_tensor_tensor(
                        vb[:], it[:, t], 1.0, it[:, t], alu.is_lt, alu.mult)
                # spikes for all T at once: s = (v_charged >= 1), bf16
                st = sp.tile([128, T, O, BS], dt.bfloat16, tag=f"s{li}")
                nc.vector.tensor_scalar(st[:], it[:], 1.0, None, alu.is_ge)
            nc.sync.dma_start(out_d.ap(), st[:])

    nc.compile()
    return nc


def _get_nc():
    if "nc" not in _CACHE:
        _CACHE["nc"] = _build_nc()
    return _CACHE["nc"]


def _host_inputs(x_tbf, Ws, bs):
    """Shared (weight/bias) arrays + per-core x shards, pre-laid-out."""
    bf16 = ml_dtypes.bfloat16
    w_arrs = []
    b_cols = []
    for li in range(4):
        W = np.asarray(Ws[li], np.float32)
        b = np.asarray(bs[li], np.float32)
        O = O_LIST[li]
        if W.shape[0] < O * 128:           # pad layer 4: 1000 -> 1024
            pad = O * 128 - W.shape[0]
            W = np.concatenate([W, np.zeros((pad, NIN), np.float32)], 0)
            b = np.concatenate([b, np.zeros(pad, np.float32)])
        # warr[o, ki, k, mo] = W[o*128+mo, k*128+ki]
        w_arrs.append(np.ascontiguousarray(
            W.reshape(O, 128, KT, 128).transpose(0, 3, 2, 1)).astype(bf16))
        b_cols.append(b.reshape(O, 128))
    b_all = np.ascontiguousarray(np.concatenate(b_cols, 0).T).astype(np.float32)

    x = np.asarray(x_tbf, np.float32)
    x_shards = []
    for c in range(NCORES):
        xc = x[:, c * BS:(c + 1) * BS, :]                    # [T, BS, NIN]
        xc = xc.transpose(2, 0, 1).reshape(NIN, COLS)        # [n, t*BS+b]
        xc = xc.reshape(KT, 128, COLS).transpose(1, 0, 2)    # [p, k, cols]
        x_shards.append(np.ascontiguousarray(xc).astype(bf16))
    return w_arrs, b_all, x_shards


def _decode_out(oc):
    """[128, T, 8, BS] (p,t,o,b) -> [T, BS, 1000] fp32."""
    oc = np.asarray(oc).astype(np.float32)
    oc = oc.transpose(1, 3, 2, 0).reshape(T, BS, O_LIST[3] * 128)
    return oc[:, :, :1000]


def kernel(x_tbf, W1, b1, W2, b2, W3, b3, W4, b4):
    global LAST_RESULTS
    from concourse.bass_utils import run_bass_kernel_spmd

    nc = _get_nc()
    w_arrs, b_all, x_shards = _host_inputs(
        x_tbf, [W1, W2, W3, W4], [b1, b2, b3, b4])

    in_maps = []
    for c in range(NCORES):
        m = {"x": x_shards[c], "bias": b_all}
        for li in range(4):
            m[f"w{li}"] = w_arrs[li]
        in_maps.append(m)

    res = run_bass_kernel_spmd(nc, in_maps, core_ids=list(range(NCORES)),
                               trace=TRACE)
    LAST_RESULTS = res

    out = np.empty((T, B, 1000), np.float32)
    for c in range(NCORES):
        out[:, c * BS:(c + 1) * BS, :] = _decode_out(res.results[c]["out"])
    return out


# revision 4
# speedup vs baseline: 3.2303x; 1.4765x over previous
"""Trainium2 Bass kernel for a 4-layer LIF spiking net (BPSpikingNet).

Reference semantics (per timestep t, per layer l):
    i = h @ W_l.T + b_l
    v = v - v/tau + i          (tau=2  ->  v = 0.5*v + i)
    s = (v >= 1.0)
    v = (1-s) * v              (hard reset to 0)
    h = s
Output = layer-4 spike train, shape [T=32, B=128, 1000], fp32.

Strategy:
  * Data-parallel over batch: B=128 -> 16 samples per core across 8 cores.
  * Layer-by-layer: layer l's matmul input (spikes of l-1) is fully known
    once l-1's recurrence is done, so each layer is ONE dense GEMM over all
    T*Bs = 512 (t,b) columns (neuron-major / weight-stationary, N=512 moving),
    followed by a 32-step elementwise LIF recurrence on [128, O*16] tiles.
  * bf16 matmuls (spikes are exact in bf16; weight rounding is far below the
    spiking threshold margin), fp32 PSUM accumulate, fp32 recurrence.
  * Recurrence: charge writes the charged potential in-place into the current
    buffer iT[:, t] (2 DVE ops per step on the serial chain), and spikes for
    ALL timesteps are extracted afterwards with a single big is_ge op.
"""

import numpy as np
import ml_dtypes

T = 32
B = 128
NCORES = 8
BS = B // NCORES          # 16 samples per core
COLS = T * BS             # 512 (t,b) columns per core
NIN = 2048
KT = NIN // 128           # 16 k-tiles (all layers have 2048 inputs)
O_LIST = [16, 16, 16, 8]  # output 128-tiles per layer (layer 4 padded 1000->1024)
BOFF = [0, 16, 32, 48]    # bias column offset per layer
NB = sum(O_LIST)          # 56 bias columns

_CACHE = {}

TRACE = False             # set True (from test.py) to capture an NTFF profile
LAST_RESULTS = None       # BassKernelResults of the most recent run
EVICT_ENGINE = "scalar"   # "scalar" (ACT Identity+bias) or "vector" fallback


def _build_nc():
    import concourse.mybir as mybir
    import concourse.tile as tile
    from concourse import bacc

    dt = mybir.dt
    alu = mybir.AluOpType

    nc = bacc.Bacc("TRN2", target_bir_lowering=False, debug=False,
                   num_devices=NCORES)

    x_d = nc.dram_tensor("x", [128, KT, COLS], dt.bfloat16, kind="ExternalInput")
    w_d = [
        nc.dram_tensor(f"w{li}", [O_LIST[li], 128, KT, 128], dt.bfloat16,
                       kind="ExternalInput")
        for li in range(4)
    ]
    b_d = nc.dram_tensor("bias", [128, NB], dt.float32, kind="ExternalInput")
    out_d = nc.dram_tensor("out", [128, T, O_LIST[3], BS], dt.bfloat16,
                           kind="ExternalOutput")

    TH = T // 2           # 16 timesteps per half
    HC = TH * BS          # 256 columns per half

    with tile.TileContext(nc) as tc:
        with (
            tc.tile_pool(name="xp", bufs=1) as xp,
            tc.tile_pool(name="sp", bufs=1) as sp,
            tc.tile_pool(name="ip", bufs=2) as ip,
            tc.tile_pool(name="wp", bufs=6) as wp,
            tc.tile_pool(name="vp", bufs=1) as vp,
            tc.tile_pool(name="bp", bufs=1) as bp,
            tc.tile_pool(name="ps", bufs=4, space="PSUM") as ps,
        ):
            # x in 4 chunks so the first matmul starts early
            xq = []
            for c in range(4):
                xc = xp.tile([128, 4, COLS], dt.bfloat16, tag=f"x{c}")
                nc.sync.dma_start(xc[:], x_d.ap()[:, 4 * c:4 * c + 4, :])
                xq.append(xc)
            bt = bp.tile([128, NB], dt.float32)
            nc.sync.dma_start(bt[:], b_d.ap())

            its = [None] * 4
            sts = [None] * 4
            vbs = [None] * 4

            def gemm_half(li, h):
                O = O_LIST[li]
                it = its[li]
                for o in range(O):
                    wt = wp.tile([128, KT, 128], dt.bfloat16, tag="wt")
                    nc.sync.dma_start(wt[:], w_d[li].ap()[o])
                    acc = ps.tile([128, HC], dt.float32, tag="acc")
                    for k in range(KT):
                        if li == 0:
                            rhs = xq[k // 4][:, k % 4, h * HC:(h + 1) * HC]
                        else:
                            rhs = sts[li - 1][:, h * TH:(h + 1) * TH, k, :]
                        nc.tensor.matmul(acc[:], wt[:, k, :], rhs,
                                         start=(k == 0), stop=(k == KT - 1))
                    # PSUM -> SBUF eviction with bias add, scattered to t-major
                    bias_ap = bt[:, BOFF[li] + o:BOFF[li] + o + 1]
                    src = acc.rearrange("p (t b) -> p t b", t=TH)
                    dst = it[:, h * TH:(h + 1) * TH, o, :]
                    if EVICT_ENGINE == "scalar":
                        nc.scalar.activation(
                            dst, src, mybir.ActivationFunctionType.Identity,
                            bias=bias_ap, scale=1.0)
                    else:
                        nc.vector.tensor_scalar(dst, src, bias_ap, None, alu.add)

            def rec_half(li, h):
                # charge in place (iT[:,t] becomes the charged potential v(t));
                # only the reset state vb carries between steps
                it, vb = its[li], vbs[li]
                for t in range(h * TH, (h + 1) * TH):
                    nc.vector.scalar_tensor_tensor(
                        it[:, t], vb[:], 0.5, it[:, t], alu.mult, alu.add)
                    nc.vector.scalar_tensor_tensor(
                        vb[:], it[:, t], 1.0, it[:, t], alu.is_lt, alu.mult)
                # spikes for this half in one op: s = (v_charged >= 1), bf16
                sl = slice(h * TH, (h + 1) * TH)
                nc.vector.tensor_scalar(
                    sts[li][:, sl], it[:, sl], 1.0, None, alu.is_ge)

            for li in range(4):
                O = O_LIST[li]
                its[li] = ip.tile([128, T, O, BS], dt.float32, tag="it",
                                  name=f"it{li}")
                sts[li] = sp.tile([128, T, O, BS], dt.bfloat16, tag=f"s{li}",
                                  name=f"s{li}")
                vbs[li] = vp.tile([128, O, BS], dt.float32, tag=f"vb{li}",
                                  name=f"vb{li}")
                nc.vector.memset(vbs[li][:], 0.0)
                # pipeline: gemm(li,h1); gemm(li,h2) || rec(li,h1);
                # next layer's gemm h1 || rec(li,h2)
                gemm_half(li, 0)
                gemm_half(li, 1)
                rec_half(li, 0)
                rec_half(li, 1)
            nc.sync.dma_start(out_d.ap(), sts[3][:])

    nc.compile()
    return nc


def _get_nc():
    if "nc" not in _CACHE:
        _CACHE["nc"] = _build_nc()
    return _CACHE["nc"]


def _host_inputs(x_tbf, Ws, bs):
    """Shared (weight/bias) arrays + per-core x shards, pre-laid-out."""
    bf16 = ml_dtypes.bfloat16
    w_arrs = []
    b_cols = []
    for li in range(4):
        W = np.asarray(Ws[li], np.float32)
        b = np.asarray(bs[li], np.float32)
        O = O_LIST[li]
        if W.shape[0] < O * 128:           # pad layer 4: 1000 -> 1024
            pad = O * 128 - W.shape[0]
            W = np.concatenate([W, np.zeros((pad, NIN), np.float32)], 0)
            b = np.concatenate([b, np.zeros(pad, np.float32)])
        # warr[o, ki, k, mo] = W[o*128+mo, k*128+ki]
        w_arrs.append(np.ascontiguousarray(
            W.reshape(O, 128, KT, 128).transpose(0, 3, 2, 1)).astype(bf16))
        b_cols.append(b.reshape(O, 128))
    b_all = np.ascontiguousarray(np.concatenate(b_cols, 0).T).astype(np.float32)

    x = np.asarray(x_tbf, np.float32)
    x_shards = []
    for c in range(NCORES):
        xc = x[:, c * BS:(c + 1) * BS, :]                    # [T, BS, NIN]
        xc = xc.transpose(2, 0, 1).reshape(NIN, COLS)        # [n, t*BS+b]
        xc = xc.reshape(KT, 128, COLS).transpose(1, 0, 2)    # [p, k, cols]
        x_shards.append(np.ascontiguousarray(xc).astype(bf16))
    return w_arrs, b_all, x_shards


def _decode_out(oc):
    """[128, T, 8, BS] (p,t,o,b) -> [T, BS, 1000] fp32."""
    oc = np.asarray(oc).astype(np.float32)
    oc = oc.transpose(1, 3, 2, 0).reshape(T, BS, O_LIST[3] * 128)
    return oc[:, :, :1000]


def kernel(x_tbf, W1, b1, W2, b2, W3, b3, W4, b4):
    global LAST_RESULTS
    from concourse.bass_utils import run_bass_kernel_spmd

    nc = _get_nc()
    w_arrs, b_all, x_shards = _host_inputs(
        x_tbf, [W1, W2, W3, W4], [b1, b2, b3, b4])

    in_maps = []
    for c in range(NCORES):
        m = {"x": x_shards[c], "bias": b_all}
        for li in range(4):
            m[f"w{li}"] = w_arrs[li]
        in_maps.append(m)

    res = run_bass_kernel_spmd(nc, in_maps, core_ids=list(range(NCORES)),
                               trace=TRACE)
    LAST_RESULTS = res

    out = np.empty((T, B, 1000), np.float32)
    for c in range(NCORES):
        out[:, c * BS:(c + 1) * BS, :] = _decode_out(res.results[c]["out"])
    return out
